# revision 3
# baseline (speedup 1.0000x reference)
"""Trainium2 Bass kernel for HierarchicalCSNet (8 groups, 256x256, G-fused chain).

Strategy: spatial row-sharding across 8 NeuronCores. Core c owns image rows
[32c, 32c+32) and recomputes shrinking halo margins locally (zero collectives).
The tiny head (strided sample conv + 1x1 upsample + block-scatter reshape) is
computed on host; everything from the first 3x3 conv onward runs on device as
fp32r tap-accumulated matmuls.

Slot grid per core: slot s in [0,56) <-> global row 32c - 12 + s. All on-chip
feature rows are stored at pitch 258 (1 zero pad col each side). Margins:
feature_m needs margin M_m = 11 - m, h_m needs H_m = M_m + 1 (H_0 = 11).
Image-edge cores zero their out-of-image margin rows via per-core mask scalars.
"""
import sys, os
import numpy as np

for _p in ("/opt/trn_rl_repo", os.path.expanduser("~/.axon_site/_ro/trn_rl_repo")):
    if os.path.isdir(_p) and _p not in sys.path:
        sys.path.append(_p)

G, BS = 8, 32
H = W = 256
PITCH = 258
NSLOT = 54          # slots [1,55) stored in F/TA (row = slot-1)
TB_BASE = 9
TB_ROWS = 38        # slots [9,47) stored in TB (row = slot-9)


def _h_range(m):
    return (1, 55) if m == 0 else (m, 56 - m)


def _fus_range(m):
    return (m + 1, 55 - m)


def _chunks():
    """(m, s0) list for h-conv tiles, in program order."""
    out = []
    for m in range(G):
        lo, hi = _h_range(m)
        for s0 in range(lo, hi, 2):
            out.append((m, s0))
    return out


_CHUNKS = _chunks()
NCHUNK = len(_CHUNKS)

_BUILT = None


def _build_program():
    import concourse.bacc as bacc
    import concourse.mybir as mybir
    import concourse.tile as tile

    f32 = mybir.dt.float32
    f32r = mybir.dt.float32r
    PRELU = mybir.ActivationFunctionType.Prelu
    COPY = mybir.ActivationFunctionType.Copy

    nc = bacc.Bacc("TRN2", target_bir_lowering=False)
    r9_d = nc.dram_tensor("r9", [NCHUNK * 9, 516], f32r, kind="ExternalInput")
    wh_d = nc.dram_tensor("wh", [9, G * 64], f32r, kind="ExternalInput")
    wf_d = nc.dram_tensor("wf", [7 * 128, 576], f32r, kind="ExternalInput")
    wt_d = nc.dram_tensor("wt", [G * 64, 1728], f32r, kind="ExternalInput")
    w5_d = nc.dram_tensor("w5", [64, G * 9], f32r, kind="ExternalInput")
    bb_d = nc.dram_tensor("bb", [64, 39], f32, kind="ExternalInput")
    aa_d = nc.dram_tensor("aa", [64, 39], f32, kind="ExternalInput")
    mm_d = nc.dram_tensor("mm", [64, 2], f32, kind="ExternalInput")
    o_d = nc.dram_tensor("o", [G, 32 * 256], f32, kind="ExternalOutput")

    with tile.TileContext(nc) as tc:
        with tc.tile_pool(name="const", bufs=1) as cst, \
             tc.tile_pool(name="big", bufs=1) as big, \
             tc.tile_pool(name="wfp", bufs=2) as wfp, \
             tc.tile_pool(name="wtp", bufs=2) as wtp, \
             tc.tile_pool(name="r9p", bufs=4) as r9p, \
             tc.tile_pool(name="o5p", bufs=2) as o5p, \
             tc.tile_pool(name="ps", bufs=6, space="PSUM") as ps, \
             tc.tile_pool(name="p5", bufs=2, space="PSUM") as p5p:

            wh_t = cst.tile([9, G * 64], f32r)
            w5_t = cst.tile([64, G * 9], f32r)
            bb_t = cst.tile([64, 39], f32)
            aa_t = cst.tile([64, 39], f32)
            mm_t = cst.tile([64, 2], f32)
            F = big.tile([128, NSLOT * PITCH], f32r)
            TA = big.tile([64, NSLOT * PITCH], f32r)
            TB = big.tile([64, TB_ROWS * PITCH], f32r)

            nc.sync.dma_start(wh_t[:], wh_d[:])
            nc.sync.dma_start(w5_t[:], w5_d[:])
            nc.sync.dma_start(bb_t[:], bb_d[:])
            nc.sync.dma_start(aa_t[:], aa_d[:])
            nc.sync.dma_start(mm_t[:], mm_d[:])
            nc.vector.memset(F[:].bitcast(f32), 0.0)
            nc.vector.memset(TA[:].bitcast(f32), 0.0)
            nc.vector.memset(TB[:].bitcast(f32), 0.0)

            Fv = F[:].rearrange("p (r x) -> p r x", x=PITCH)
            TAv = TA[:].rearrange("p (r x) -> p r x", x=PITCH)
            TBv = TB[:].rearrange("p (r x) -> p r x", x=PITCH)

            def mask(view, base, mlo, mhi, both64=False):
                # zero out-of-image rows: top slots [mlo,12) with mm[:,0],
                # bottom slots [44,mhi) with mm[:,1] (no-op on interior cores)
                for (lo, hi, col) in ((mlo, 12, 0), (44, mhi, 1)):
                    if hi <= lo:
                        continue
                    for p0, p1 in (((0, 128),) if both64 else ((0, 64),)):
                        nc.vector.tensor_scalar_mul(
                            view[p0:p1, lo - base:hi - base, :],
                            view[p0:p1, lo - base:hi - base, :],
                            mm_t[0:64, col:col + 1])

            chunk_idx = 0
            for m in range(G):
                # --- stream this group's tail weights ---
                wt_t = wtp.tile([64, 1728], f32r, tag="wt")
                nc.sync.dma_start(wt_t[:], wt_d[m * 64:(m + 1) * 64, :])

                # --- h_m: K=9 matmuls from streamed r9 chunks ---
                h_lo, h_hi = _h_range(m)
                hdst, hbase = (TAv, 1) if m == 0 else (Fv, 1)
                for s0 in range(h_lo, h_hi, 2):
                    r9c = r9p.tile([9, 2 * 258], f32r, tag="r9")
                    nc.sync.dma_start(
                        r9c[:], r9_d[chunk_idx * 9:(chunk_idx + 1) * 9, :])
                    chunk_idx += 1
                    r9v = r9c[:].rearrange("p (r x) -> p r x", x=258)
                    pt = ps.tile([64, 512], f32, tag="ps")
                    nc.tensor.matmul(pt[:], wh_t[:, m * 64:(m + 1) * 64],
                                     r9v[0:9, 0:2, 1:257], start=True, stop=True)
                    nc.scalar.activation(
                        hdst[0:64, s0 - hbase:s0 - hbase + 2, 1:257], pt[:],
                        PRELU, bias=bb_t[:, m:m + 1], scale=1.0,
                        alpha=aa_t[:, m:m + 1])
                mask(hdst, hbase, h_lo, h_hi)

                # --- fusion m (m>=1): K=128 from F = [h_m | feature_{m-1}] ---
                if m >= 1:
                    wf_t = wfp.tile([128, 576], f32r, tag="wf")
                    nc.sync.dma_start(
                        wf_t[:], wf_d[(m - 1) * 128:m * 128, :])
                    f_lo, f_hi = _fus_range(m)
                    for s0 in range(f_lo, f_hi, 2):
                        pt = ps.tile([64, 512], f32, tag="ps")
                        for t in range(9):
                            dy, dx = t // 3, t % 3
                            rr = s0 + dy - 1 - 1
                            nc.tensor.matmul(
                                pt[:], wf_t[:, t * 64:(t + 1) * 64],
                                Fv[0:128, rr:rr + 2, dx:dx + 256],
                                start=(t == 0), stop=(t == 8))
                        nc.scalar.activation(
                            TAv[0:64, s0 - 1:s0 + 1, 1:257], pt[:],
                            PRELU, bias=bb_t[:, 8 + m - 1:8 + m],
                            scale=1.0, alpha=aa_t[:, 8 + m - 1:8 + m])
                    mask(TAv, 1, f_lo, f_hi)

                # --- feature_m (in TA) -> F[64:128] for next fusion ---
                if m < G - 1:
                    lo, hi = (1, 55) if m == 0 else _fus_range(m)
                    nc.sync.dma_start(
                        F[64:128, (lo - 1) * PITCH:(hi - 1) * PITCH],
                        TA[0:64, (lo - 1) * PITCH:(hi - 1) * PITCH])

                # --- tails ---
                def tconv(src_v, src_base, dst_v, dst_base, lo, hi, wcol_base,
                          bcol):
                    for s0 in range(lo, hi, 2):
                        pt = ps.tile([64, 512], f32, tag="ps")
                        for t in range(9):
                            dy, dx = t // 3, t % 3
                            rr = s0 + dy - 1 - src_base
                            nc.tensor.matmul(
                                pt[:],
                                wt_t[:, (wcol_base + t) * 64:(wcol_base + t + 1) * 64],
                                src_v[0:64, rr:rr + 2, dx:dx + 256],
                                start=(t == 0), stop=(t == 8))
                        nc.scalar.activation(
                            dst_v[0:64, s0 - dst_base:s0 - dst_base + 2, 1:257],
                            pt[:], PRELU, bias=bb_t[:, bcol:bcol + 1],
                            scale=1.0, alpha=aa_t[:, bcol:bcol + 1])

                tconv(TAv, 1, TBv, TB_BASE, 9, 47, 0, 15 + m)    # t2
                mask(TBv, TB_BASE, 9, 47)
                tconv(TBv, TB_BASE, TAv, 1, 10, 46, 9, 23 + m)   # t3
                mask(TAv, 1, 10, 46)
                tconv(TAv, 1, TBv, TB_BASE, 11, 45, 18, 31 + m)  # t4
                mask(TBv, TB_BASE, 11, 45)

                # --- t5: K=64, M=1 ---
                for s0 in range(12, 44, 2):
                    pt5 = p5p.tile([1, 512], f32, tag="p5")
                    for t in range(9):
                        dy, dx = t // 3, t % 3
                        rr = s0 + dy - 1 - TB_BASE
                        nc.tensor.matmul(
                            pt5[:], w5_t[:, m * 9 + t:m * 9 + t + 1],
                            TBv[0:64, rr:rr + 2, dx:dx + 256],
                            start=(t == 0), stop=(t == 8))
                    o5 = o5p.tile([1, 512], f32, tag="o5")
                    nc.scalar.activation(o5[:], pt5[:], COPY)
                    nc.sync.dma_start(
                        o_d[m, (s0 - 12) * 256:(s0 - 10) * 256], o5[:])

    nc.compile()
    return nc


def _get_program():
    global _BUILT
    if _BUILT is None:
        _BUILT = _build_program()
    return _BUILT


def _host_heads(x, sample_w, up_w, up_b):
    """r[m] (256x256) for all groups, float32."""
    X = x[0, 0].reshape(8, 32, 8, 32).astype(np.float64)
    R = np.empty((G, H, W), np.float32)
    for m in range(G):
        S = np.einsum('ipjq,cpq->cij', X, sample_w[m, :, 0].astype(np.float64))
        U = np.einsum('cij,uc->uij', S, up_w[m, :, :, 0, 0].astype(np.float64))
        U = U + up_b[m].astype(np.float64)[:, None, None]
        R[m] = U.reshape(32, 32, 8, 8).transpose(2, 0, 3, 1).reshape(256, 256)
    return R


def _build_r9(R):
    """Per-core prestacked h-conv rhs: [8][NCHUNK*9, 516] float32."""
    rp = np.zeros((G, H + 26, W + 4), np.float32)   # rows g+13, cols x+2
    rp[:, 13:13 + H, 2:2 + W] = R
    out = np.empty((8, NCHUNK * 9, 516), np.float32)
    for c in range(8):
        k = 0
        for (m, s0) in _CHUNKS:
            for t in range(9):
                dy, dx = t // 3, t % 3
                g0 = 32 * c + s0 + dy
                out[c, k * 9 + t] = rp[m, g0:g0 + 2, dx:dx + 258].reshape(516)
            k += 1
    return out


def kernel(x, sample_w, up_w, up_b, h1_w, h1_b, h1_a, fus_w, fus_b, fus_a,
           t2_w, t2_b, t2_a, t3_w, t3_b, t3_a, t4_w, t4_b, t4_a, t5_w, t5_b):
    from concourse import bass2jax

    nc = _get_program()

    R = _host_heads(x, sample_w, up_w, up_b)
    r9 = _build_r9(R)

    wh = np.ascontiguousarray(
        h1_w[:, :, 0].reshape(G, 64, 9).transpose(2, 0, 1).reshape(9, G * 64))
    # fusion lhsT rows 0:64 <- h weights (cat idx 64:128), rows 64:128 <- feature
    wf = np.empty((7, 128, 9, 64), np.float32)
    for mm1 in range(7):
        for t in range(9):
            wf[mm1, 0:64, t] = fus_w[mm1, :, 64:128, t // 3, t % 3].T
            wf[mm1, 64:128, t] = fus_w[mm1, :, 0:64, t // 3, t % 3].T
    wf = wf.reshape(7 * 128, 576)
    wt = np.empty((G, 64, 27, 64), np.float32)
    for m in range(G):
        for i, tw in enumerate((t2_w, t3_w, t4_w)):
            for t in range(9):
                wt[m, :, i * 9 + t] = tw[m, :, :, t // 3, t % 3].T
    wt = wt.reshape(G * 64, 1728)
    w5 = np.empty((64, G * 9), np.float32)
    for m in range(G):
        for t in range(9):
            w5[:, m * 9 + t] = t5_w[m, 0, :, t // 3, t % 3]
    bb = np.zeros((64, 39), np.float32)
    aa = np.zeros((64, 39), np.float32)
    bb[:, 0:8] = h1_b.T; aa[:, 0:8] = np.broadcast_to(h1_a, (64, 8))
    bb[:, 8:15] = fus_b.T; aa[:, 8:15] = np.broadcast_to(fus_a, (64, 7))
    bb[:, 15:23] = t2_b.T; aa[:, 15:23] = np.broadcast_to(t2_a, (64, 8))
    bb[:, 23:31] = t3_b.T; aa[:, 23:31] = np.broadcast_to(t3_a, (64, 8))
    bb[:, 31:39] = t4_b.T; aa[:, 31:39] = np.broadcast_to(t4_a, (64, 8))

    in_maps = []
    for c in range(8):
        mmk = np.ones((64, 2), np.float32)
        if c == 0:
            mmk[:, 0] = 0.0
        if c == 7:
            mmk[:, 1] = 0.0
        in_maps.append({"r9": r9[c], "wh": wh, "wf": wf, "wt": wt, "w5": w5,
                        "bb": bb, "aa": aa, "mm": mmk})

    results = bass2jax.run_bass_via_pjrt(nc, in_maps, n_cores=8)

    out = np.empty((G, 1, 1, H, W), np.float32)
    for c in range(8):
        o = results[c]["o"].reshape(G, 32, 256)
        out[:, 0, 0, 32 * c:32 * c + 32, :] = o
    out += t5_b.reshape(G, 1, 1, 1, 1)
    return out


# revision 16
# speedup vs baseline: 24.8463x; 24.8463x over previous
"""Trainium2 Bass kernel for HierarchicalCSNet (8 groups, 256x256, G-fused chain).

Strategy: spatial row-sharding across 8 NeuronCores. Core c owns image rows
[32c, 32c+32) and recomputes shrinking halo margins locally (zero collectives).
The tiny head (strided sample conv + 1x1 upsample + block-scatter reshape) is
computed on host; everything from the first 3x3 conv onward runs on device as
fp32r tap-accumulated matmuls.

Slot grid per core: slot s in [0,56) <-> global row 32c - 12 + s. All on-chip
feature rows are stored at pitch 258 (1 zero pad col each side). Margins:
feature_m needs margin M_m = 11 - m, h_m needs H_m = M_m + 1 (H_0 = 11).
Image-edge cores zero their out-of-image margin rows via per-core mask scalars.
"""
import sys, os
import numpy as np

for _p in ("/opt/trn_rl_repo", os.path.expanduser("~/.axon_site/_ro/trn_rl_repo")):
    if os.path.isdir(_p) and _p not in sys.path:
        sys.path.append(_p)

G, BS = 8, 32
H = W = 256
PITCH = 258
NSLOT = 54          # slots [1,55) stored in F/TA (row = slot-1)
TB_BASE = 9
TB_ROWS = 38        # slots [9,47) stored in TB (row = slot-9)


def _h_range(m):
    return (1, 55) if m == 0 else (m, 56 - m)


def _fus_range(m):
    return (m + 1, 55 - m)


def _chunks():
    """(m, s0) list for h-conv tiles, in program order."""
    out = []
    for m in range(G):
        lo, hi = _h_range(m)
        for s0 in range(lo, hi, 2):
            out.append((m, s0))
    return out


_CHUNKS = _chunks()
NCHUNK = len(_CHUNKS)

_BUILT = None


def _build_program():
    import concourse.bacc as bacc
    import concourse.mybir as mybir
    import concourse.tile as tile

    f32 = mybir.dt.float32
    f32r = mybir.dt.float32r
    PRELU = mybir.ActivationFunctionType.Prelu
    COPY = mybir.ActivationFunctionType.Copy

    nc = bacc.Bacc("TRN2", target_bir_lowering=False)
    r9_d = nc.dram_tensor("r9", [NCHUNK * 9, 516], f32r, kind="ExternalInput")
    wh_d = nc.dram_tensor("wh", [9, G * 64], f32r, kind="ExternalInput")
    wf_d = nc.dram_tensor("wf", [7 * 128, 576], f32r, kind="ExternalInput")
    wt_d = nc.dram_tensor("wt", [G * 128, 1152], f32r, kind="ExternalInput")
    w5_d = nc.dram_tensor("w5", [128, G * 6], f32r, kind="ExternalInput")
    bb_d = nc.dram_tensor("bb", [64, 39], f32, kind="ExternalInput")
    aa_d = nc.dram_tensor("aa", [64, 39], f32, kind="ExternalInput")
    mm_d = nc.dram_tensor("mm", [128, 2], f32, kind="ExternalInput")
    o_d = nc.dram_tensor("o", [G, 32 * 256], f32, kind="ExternalOutput")

    with tile.TileContext(nc) as tc:
        with tc.tile_pool(name="const", bufs=1) as cst, \
             tc.tile_pool(name="big", bufs=1) as big, \
             tc.tile_pool(name="wfp", bufs=2) as wfp, \
             tc.tile_pool(name="wtp", bufs=2) as wtp, \
             tc.tile_pool(name="r9p", bufs=4) as r9p, \
             tc.tile_pool(name="o5p", bufs=2) as o5p, \
             tc.tile_pool(name="ps", bufs=6, space="PSUM") as ps, \
             tc.tile_pool(name="p5", bufs=2, space="PSUM") as p5p:

            wh_t = cst.tile([9, G * 64], f32r)
            w5_t = cst.tile([128, G * 6], f32r)
            bb_t = cst.tile([64, 39], f32)
            aa_t = cst.tile([64, 39], f32)
            mm_t = cst.tile([128, 2], f32)
            F = big.tile([128, NSLOT * PITCH], f32r)
            TA = big.tile([128, NSLOT * PITCH], f32r)
            TB = big.tile([128, TB_ROWS * PITCH], f32r)

            nc.sync.dma_start(wh_t[:], wh_d[:])
            nc.sync.dma_start(w5_t[:], w5_d[:])
            nc.sync.dma_start(bb_t[:], bb_d[:])
            nc.sync.dma_start(aa_t[:], aa_d[:])
            nc.sync.dma_start(mm_t[:], mm_d[:])
            nc.vector.memset(F[:].bitcast(f32), 0.0)
            nc.vector.memset(TA[:].bitcast(f32), 0.0)
            nc.vector.memset(TB[:].bitcast(f32), 0.0)

            Fv = F[:].rearrange("p (r x) -> p r x", x=PITCH)
            TAv = TA[:].rearrange("p (r x) -> p r x", x=PITCH)
            TBv = TB[:].rearrange("p (r x) -> p r x", x=PITCH)

            def mask(view, base, mlo, mhi, stacked=False, nrows=NSLOT):
                # zero out-of-image rows: top slots [mlo,12) with mm[:,0],
                # bottom slots [44,mhi) with mm[:,1] (no-op on interior cores)
                for (lo, hi, col) in ((mlo, 12, 0), (44, mhi, 1)):
                    if hi <= lo:
                        continue
                    nc.vector.tensor_scalar_mul(
                        view[0:64, lo - base:hi - base, :],
                        view[0:64, lo - base:hi - base, :],
                        mm_t[0:64, col:col + 1])
                if not stacked:
                    return
                # upper half holds rows shifted by +1 slot
                for (lo, hi, col) in ((mlo, 12, 0), (44, mhi, 1)):
                    rlo = max(0, lo - base - 1)
                    rhi = min(nrows, hi - base - 1)
                    if rhi <= rlo:
                        continue
                    nc.vector.tensor_scalar_mul(
                        view[64:128, rlo:rhi, :],
                        view[64:128, rlo:rhi, :],
                        mm_t[64:128, col:col + 1])

            def stack_dma(buf, base, s0, nrows):
                # buf[64:128, r] := buf[0:64, r+1] for the rows enabled by the
                # freshly written tile (slots s0, s0+1)
                d0 = max(0, s0 - base - 1)
                d1 = min(nrows - 1, s0 - base + 1)
                if d1 <= d0:
                    return
                nc.sync.dma_start(
                    buf[64:128, d0 * PITCH:d1 * PITCH],
                    buf[0:64, (d0 + 1) * PITCH:(d1 + 1) * PITCH])

            chunk_idx = 0
            for m in range(G):
                # --- stream this group's tail weights ---
                wt_t = wtp.tile([128, 1152], f32r, tag="wt")
                nc.sync.dma_start(wt_t[:], wt_d[m * 128:(m + 1) * 128, :])

                # --- h_m: K=9 matmuls from streamed r9 chunks ---
                h_lo, h_hi = _h_range(m)
                hdst, hbase = (TAv, 1) if m == 0 else (Fv, 1)
                for s0 in range(h_lo, h_hi, 2):
                    r9c = r9p.tile([9, 2 * 258], f32r, tag="r9")
                    nc.sync.dma_start(
                        r9c[:], r9_d[chunk_idx * 9:(chunk_idx + 1) * 9, :])
                    chunk_idx += 1
                    r9v = r9c[:].rearrange("p (r x) -> p r x", x=258)
                    pt = ps.tile([64, 512], f32, tag="ps")
                    nc.tensor.matmul(pt[:], wh_t[:, m * 64:(m + 1) * 64],
                                     r9v[0:9, 0:2, 1:257], start=True, stop=True)
                    nc.scalar.activation(
                        hdst[0:64, s0 - hbase:s0 - hbase + 2, 1:257], pt[:],
                        PRELU, bias=bb_t[:, m:m + 1], scale=1.0,
                        alpha=aa_t[:, m:m + 1])
                    if m == 0:
                        stack_dma(TA, 1, s0, NSLOT)
                mask(hdst, hbase, h_lo, h_hi, stacked=(m == 0))

                # --- fusion m (m>=1): K=128 from F = [h_m | feature_{m-1}] ---
                if m >= 1:
                    wf_t = wfp.tile([128, 576], f32r, tag="wf")
                    nc.sync.dma_start(
                        wf_t[:], wf_d[(m - 1) * 128:m * 128, :])
                    f_lo, f_hi = _fus_range(m)
                    for s0 in range(f_lo, f_hi, 2):
                        pt = ps.tile([64, 512], f32, tag="ps")
                        for t in range(9):
                            dy, dx = t // 3, t % 3
                            rr = s0 + dy - 1 - 1
                            nc.tensor.matmul(
                                pt[:], wf_t[:, t * 64:(t + 1) * 64],
                                Fv[0:128, rr:rr + 2, dx:dx + 256],
                                start=(t == 0), stop=(t == 8))
                        nc.scalar.activation(
                            TAv[0:64, s0 - 1:s0 + 1, 1:257], pt[:],
                            PRELU, bias=bb_t[:, 8 + m - 1:8 + m],
                            scale=1.0, alpha=aa_t[:, 8 + m - 1:8 + m])
                        stack_dma(TA, 1, s0, NSLOT)
                    mask(TAv, 1, f_lo, f_hi, stacked=True)

                # --- feature_m (in TA) -> F[64:128] for next fusion ---
                if m < G - 1:
                    lo, hi = (1, 55) if m == 0 else _fus_range(m)
                    nc.sync.dma_start(
                        F[64:128, (lo - 1) * PITCH:(hi - 1) * PITCH],
                        TA[0:64, (lo - 1) * PITCH:(hi - 1) * PITCH])

                # --- tails (dy-packed: 3x K=128 + 3x K=64 per tile) ---
                def tconv(src_v, src_base, dst_v, dst_base, dst_buf, dst_rows,
                          lo, hi, cv, bcol):
                    for s0 in range(lo, hi, 2):
                        pt = ps.tile([64, 512], f32, tag="ps")
                        for j in range(6):
                            dx = j % 3
                            c0 = (cv * 6 + j) * 64
                            if j < 3:   # dy=0 (lower) + dy=1 (stacked upper)
                                rr = s0 - 1 - src_base
                                nc.tensor.matmul(
                                    pt[:], wt_t[:, c0:c0 + 64],
                                    src_v[0:128, rr:rr + 2, dx:dx + 256],
                                    start=(j == 0), stop=False)
                            else:       # dy=2 from lower half
                                rr = s0 + 1 - src_base
                                nc.tensor.matmul(
                                    pt[:], wt_t[0:64, c0:c0 + 64],
                                    src_v[0:64, rr:rr + 2, dx:dx + 256],
                                    start=False, stop=(j == 5))
                        nc.scalar.activation(
                            dst_v[0:64, s0 - dst_base:s0 - dst_base + 2, 1:257],
                            pt[:], PRELU, bias=bb_t[:, bcol:bcol + 1],
                            scale=1.0, alpha=aa_t[:, bcol:bcol + 1])
                        stack_dma(dst_buf, dst_base, s0, dst_rows)

                tconv(TAv, 1, TBv, TB_BASE, TB, TB_ROWS, 9, 47, 0, 15 + m)
                mask(TBv, TB_BASE, 9, 47, stacked=True, nrows=TB_ROWS)
                tconv(TBv, TB_BASE, TAv, 1, TA, NSLOT, 10, 46, 1, 23 + m)
                mask(TAv, 1, 10, 46, stacked=True)
                tconv(TAv, 1, TBv, TB_BASE, TB, TB_ROWS, 11, 45, 2, 31 + m)
                mask(TBv, TB_BASE, 11, 45, stacked=True, nrows=TB_ROWS)

                # --- t5: M=1, dy-packed like the tails ---
                for s0 in range(12, 44, 2):
                    pt5 = p5p.tile([1, 512], f32, tag="p5")
                    for j in range(6):
                        dx = j % 3
                        c5 = m * 6 + j
                        if j < 3:
                            rr = s0 - 1 - TB_BASE
                            nc.tensor.matmul(
                                pt5[:], w5_t[:, c5:c5 + 1],
                                TBv[0:128, rr:rr + 2, dx:dx + 256],
                                start=(j == 0), stop=False)
                        else:
                            rr = s0 + 1 - TB_BASE
                            nc.tensor.matmul(
                                pt5[:], w5_t[0:64, c5:c5 + 1],
                                TBv[0:64, rr:rr + 2, dx:dx + 256],
                                start=False, stop=(j == 5))
                    o5 = o5p.tile([1, 512], f32, tag="o5")
                    nc.scalar.activation(o5[:], pt5[:], COPY)
                    nc.sync.dma_start(
                        o_d[m, (s0 - 12) * 256:(s0 - 10) * 256], o5[:])

    nc.compile()
    return nc


def _get_program():
    global _BUILT
    if _BUILT is None:
        _BUILT = _build_program()
    return _BUILT


def _host_heads(x, sample_w, up_w, up_b):
    """r[m] (256x256) for all groups, float32."""
    X = x[0, 0].reshape(8, 32, 8, 32).astype(np.float64)
    R = np.empty((G, H, W), np.float32)
    for m in range(G):
        S = np.einsum('ipjq,cpq->cij', X, sample_w[m, :, 0].astype(np.float64))
        U = np.einsum('cij,uc->uij', S, up_w[m, :, :, 0, 0].astype(np.float64))
        U = U + up_b[m].astype(np.float64)[:, None, None]
        R[m] = U.reshape(32, 32, 8, 8).transpose(2, 0, 3, 1).reshape(256, 256)
    return R


def _build_r9(R):
    """Per-core prestacked h-conv rhs: [8][NCHUNK*9, 516] float32."""
    from numpy.lib.stride_tricks import sliding_window_view
    rp = np.zeros((G, H + 26, W + 4), np.float32)   # rows g+13, cols x+2
    rp[:, 13:13 + H, 2:2 + W] = R
    out = np.empty((8, NCHUNK, 9, 516), np.float32)
    k0 = 0
    for m in range(G):
        lo, hi = _h_range(m)
        s0s = np.arange(lo, hi, 2)
        SW = sliding_window_view(rp[m], (2, 258))
        for t in range(9):
            dy, dx = t // 3, t % 3
            g0 = (32 * np.arange(8))[:, None] + s0s[None, :] + dy
            out[:, k0:k0 + len(s0s), t] = SW[g0, dx].reshape(8, len(s0s), 516)
        k0 += len(s0s)
    return out.reshape(8, NCHUNK * 9, 516)


_EXEC = None


def _get_executor():
    """Persistent jitted shard_map executor over 8 cores (mirrors
    bass2jax.run_bass_via_pjrt, but reusable for repeat timing)."""
    global _EXEC
    if _EXEC is not None:
        return _EXEC
    import jax
    import jax.numpy as jnp
    from jax.sharding import Mesh, PartitionSpec
    from jax.experimental.shard_map import shard_map
    import concourse.mybir as mybir
    from concourse import bass2jax

    nc = _get_program()
    bass2jax.install_neuronx_cc_hook()

    part_name = nc.partition_id_tensor.name if nc.partition_id_tensor else None
    in_names, out_names, out_avals, zero_shapes = [], [], [], []
    for alloc in nc.m.functions[0].allocations:
        if not isinstance(alloc, mybir.MemoryLocationSet):
            continue
        name = alloc.memorylocations[0].name
        if alloc.kind == "ExternalInput":
            if name != part_name:
                in_names.append(name)
        elif alloc.kind == "ExternalOutput":
            out_names.append(name)
            shape = tuple(alloc.tensor_shape)
            dtype = mybir.dt.np(alloc.dtype)
            out_avals.append(jax.core.ShapedArray(shape, dtype))
            zero_shapes.append((shape, dtype))
    n_params = len(in_names)
    all_names = in_names + out_names
    if part_name is not None:
        all_names = all_names + [part_name]

    def _body(*args):
        operands = list(args)
        if part_name is not None:
            operands.append(bass2jax.partition_id_tensor())
        outs = bass2jax._bass_exec_p.bind(
            *operands,
            out_avals=tuple(out_avals),
            in_names=tuple(all_names),
            out_names=tuple(out_names),
            lowering_input_output_aliases=(),
            sim_require_finite=True,
            sim_require_nnan=True,
            nc=nc,
        )
        return tuple(outs)

    devices = jax.devices()[:8]
    mesh = Mesh(np.asarray(devices), ("core",))
    n_outs = len(out_names)
    sharded = jax.jit(
        shard_map(_body, mesh=mesh,
                  in_specs=(PartitionSpec("core"),) * (n_params + n_outs),
                  out_specs=(PartitionSpec("core"),) * n_outs,
                  check_rep=False),
        keep_unused=True)
    _EXEC = (sharded, in_names, out_names, zero_shapes)
    return _EXEC


def _prep_device_args(in_maps):
    import jax
    sharded, in_names, out_names, zero_shapes = _get_executor()
    concat_in = [np.concatenate([in_maps[c][n] for c in range(8)], axis=0)
                 for n in in_names]
    concat_zero = [np.zeros((8 * s[0],) + tuple(s[1:]), d)
                   for (s, d) in zero_shapes]
    return [jax.device_put(a) for a in concat_in + concat_zero]


def _run(in_maps):
    sharded, in_names, out_names, zero_shapes = _get_executor()
    args = _prep_device_args(in_maps)
    outs = sharded(*args)
    res = []
    for c in range(8):
        res.append({n: np.asarray(outs[i]).reshape((8,) + zero_shapes[i][0])[c]
                    for i, n in enumerate(out_names)})
    return res


def bench(in_maps, iters=5):
    """Device-resident repeat timing of the sharded program. Returns
    (best_seconds, times)."""
    import time as _t
    sharded, *_ = _get_executor()
    args = _prep_device_args(in_maps)
    r = sharded(*args)
    [x.block_until_ready() for x in r]
    times = []
    for _ in range(iters):
        t0 = _t.perf_counter()
        r = sharded(*args)
        [x.block_until_ready() for x in r]
        times.append(_t.perf_counter() - t0)
    return min(times), times


def build_in_maps(x, sample_w, up_w, up_b, h1_w, h1_b, h1_a, fus_w, fus_b,
                  fus_a, t2_w, t2_b, t2_a, t3_w, t3_b, t3_a, t4_w, t4_b,
                  t4_a, t5_w, t5_b):

    R = _host_heads(x, sample_w, up_w, up_b)
    r9 = _build_r9(R)

    wh = np.ascontiguousarray(
        h1_w[:, :, 0].reshape(G, 64, 9).transpose(2, 0, 1).reshape(9, G * 64))
    # fusion lhsT rows 0:64 <- h weights (cat idx 64:128), rows 64:128 <- feature
    wf = np.empty((7, 128, 9, 64), np.float32)
    for mm1 in range(7):
        for t in range(9):
            wf[mm1, 0:64, t] = fus_w[mm1, :, 64:128, t // 3, t % 3].T
            wf[mm1, 64:128, t] = fus_w[mm1, :, 0:64, t // 3, t % 3].T
    wf = wf.reshape(7 * 128, 576)
    wt = np.zeros((G, 128, 3, 6, 64), np.float32)
    for m in range(G):
        for cv, tw in enumerate((t2_w, t3_w, t4_w)):
            for dx in range(3):
                wt[m, 0:64, cv, dx] = tw[m, :, :, 0, dx].T
                wt[m, 64:128, cv, dx] = tw[m, :, :, 1, dx].T
                wt[m, 0:64, cv, 3 + dx] = tw[m, :, :, 2, dx].T
    wt = wt.reshape(G * 128, 1152)
    w5 = np.zeros((128, G * 6), np.float32)
    for m in range(G):
        for dx in range(3):
            w5[0:64, m * 6 + dx] = t5_w[m, 0, :, 0, dx]
            w5[64:128, m * 6 + dx] = t5_w[m, 0, :, 1, dx]
            w5[0:64, m * 6 + 3 + dx] = t5_w[m, 0, :, 2, dx]
    bb = np.zeros((64, 39), np.float32)
    aa = np.zeros((64, 39), np.float32)
    bb[:, 0:8] = h1_b.T; aa[:, 0:8] = np.broadcast_to(h1_a, (64, 8))
    bb[:, 8:15] = fus_b.T; aa[:, 8:15] = np.broadcast_to(fus_a, (64, 7))
    bb[:, 15:23] = t2_b.T; aa[:, 15:23] = np.broadcast_to(t2_a, (64, 8))
    bb[:, 23:31] = t3_b.T; aa[:, 23:31] = np.broadcast_to(t3_a, (64, 8))
    bb[:, 31:39] = t4_b.T; aa[:, 31:39] = np.broadcast_to(t4_a, (64, 8))

    in_maps = []
    for c in range(8):
        mmk = np.ones((128, 2), np.float32)
        if c == 0:
            mmk[:, 0] = 0.0
        if c == 7:
            mmk[:, 1] = 0.0
        in_maps.append({"r9": r9[c], "wh": wh, "wf": wf, "wt": wt, "w5": w5,
                        "bb": bb, "aa": aa, "mm": mmk})
    return in_maps


def kernel(x, sample_w, up_w, up_b, h1_w, h1_b, h1_a, fus_w, fus_b, fus_a,
           t2_w, t2_b, t2_a, t3_w, t3_b, t3_a, t4_w, t4_b, t4_a, t5_w, t5_b):
    in_maps = build_in_maps(
        x, sample_w, up_w, up_b, h1_w, h1_b, h1_a, fus_w, fus_b, fus_a,
        t2_w, t2_b, t2_a, t3_w, t3_b, t3_a, t4_w, t4_b, t4_a, t5_w, t5_b)
    results = _run(in_maps)
    out = np.empty((G, 1, 1, H, W), np.float32)
    for c in range(8):
        o = results[c]["o"].reshape(G, 32, 256)
        out[:, 0, 0, 32 * c:32 * c + 32, :] = o
    out += np.asarray(t5_b).reshape(G, 1, 1, 1, 1)
    return out


# revision 20
# speedup vs baseline: 35.8185x; 1.4416x over previous
"""Trainium2 Bass kernel for HierarchicalCSNet (8 groups, 256x256, G-fused chain).

Strategy: spatial row-sharding across 8 NeuronCores. Core c owns image rows
[32c, 32c+32) and recomputes shrinking halo margins locally (zero collectives).
The tiny head (strided sample conv + 1x1 upsample + block-scatter reshape) is
computed on host; everything from the first 3x3 conv onward runs on device as
fp32r tap-accumulated matmuls.

Slot grid per core: slot s in [0,56) <-> global row 32c - 12 + s. All on-chip
feature rows are stored at pitch 258 (1 zero pad col each side). Margins:
feature_m needs margin M_m = 11 - m, h_m needs H_m = M_m + 1 (H_0 = 11).
Image-edge cores zero their out-of-image margin rows via per-core mask scalars.
"""
import sys, os
import numpy as np

for _p in ("/opt/trn_rl_repo", os.path.expanduser("~/.axon_site/_ro/trn_rl_repo")):
    if os.path.isdir(_p) and _p not in sys.path:
        sys.path.append(_p)

G, BS = 8, 32
H = W = 256
PITCH = 258
NSLOT = 54          # slots [1,55) stored in F/TA (row = slot-1)
TB_BASE = 9
TB_ROWS = 38        # slots [9,47) stored in TB (row = slot-9)


def _h_range(m):
    return (1, 55) if m == 0 else (m, 56 - m)


def _fus_range(m):
    return (m + 1, 55 - m)


def _chunks():
    """(m, s0) list for h-conv tiles, in program order."""
    out = []
    for m in range(G):
        lo, hi = _h_range(m)
        for s0 in range(lo, hi, 2):
            out.append((m, s0))
    return out


_CHUNKS = _chunks()
NCHUNK = len(_CHUNKS)

_BUILT = None


def _build_program(reps=1):
    import concourse.bacc as bacc
    import concourse.mybir as mybir
    import concourse.tile as tile

    f32 = mybir.dt.float32
    f32r = mybir.dt.float32r
    PRELU = mybir.ActivationFunctionType.Prelu
    COPY = mybir.ActivationFunctionType.Copy

    nc = bacc.Bacc("TRN2", target_bir_lowering=False)
    r9_d = nc.dram_tensor("r9", [NCHUNK * 9, 516], f32r, kind="ExternalInput")
    wh_d = nc.dram_tensor("wh", [9, G * 64], f32r, kind="ExternalInput")
    wf_d = nc.dram_tensor("wf", [7 * 128, 576], f32r, kind="ExternalInput")
    wt_d = nc.dram_tensor("wt", [G * 128, 1152], f32r, kind="ExternalInput")
    w5_d = nc.dram_tensor("w5", [128, G * 6], f32r, kind="ExternalInput")
    bb_d = nc.dram_tensor("bb", [64, 39], f32, kind="ExternalInput")
    aa_d = nc.dram_tensor("aa", [64, 39], f32, kind="ExternalInput")
    mm_d = nc.dram_tensor("mm", [128, 2], f32, kind="ExternalInput")
    o_d = nc.dram_tensor("o", [G, 32 * 256], f32, kind="ExternalOutput")

    with tile.TileContext(nc) as tc:
        with tc.tile_pool(name="const", bufs=1) as cst, \
             tc.tile_pool(name="big", bufs=1) as big, \
             tc.tile_pool(name="wfp", bufs=2) as wfp, \
             tc.tile_pool(name="wtp", bufs=2) as wtp, \
             tc.tile_pool(name="r9p", bufs=4) as r9p, \
             tc.tile_pool(name="o5p", bufs=2) as o5p, \
             tc.tile_pool(name="ps", bufs=6, space="PSUM") as ps, \
             tc.tile_pool(name="p5", bufs=2, space="PSUM") as p5p:

            wh_t = cst.tile([9, G * 64], f32r)
            w5_t = cst.tile([128, G * 6], f32r)
            bb_t = cst.tile([64, 39], f32)
            aa_t = cst.tile([64, 39], f32)
            mm_t = cst.tile([128, 2], f32)
            F = big.tile([128, NSLOT * PITCH], f32r)
            TA = big.tile([128, NSLOT * PITCH], f32r)
            TB = big.tile([128, TB_ROWS * PITCH], f32r)

            nc.sync.dma_start(wh_t[:], wh_d[:])
            nc.sync.dma_start(w5_t[:], w5_d[:])
            nc.sync.dma_start(bb_t[:], bb_d[:])
            nc.sync.dma_start(aa_t[:], aa_d[:])
            nc.sync.dma_start(mm_t[:], mm_d[:])
            nc.vector.memset(F[:].bitcast(f32), 0.0)
            nc.vector.memset(TA[:].bitcast(f32), 0.0)
            nc.vector.memset(TB[:].bitcast(f32), 0.0)

            Fv = F[:].rearrange("p (r x) -> p r x", x=PITCH)
            TAv = TA[:].rearrange("p (r x) -> p r x", x=PITCH)
            TBv = TB[:].rearrange("p (r x) -> p r x", x=PITCH)

            def mask(view, base, mlo, mhi, stacked=False, nrows=NSLOT):
                # zero out-of-image rows: top slots [mlo,12) with mm[:,0],
                # bottom slots [44,mhi) with mm[:,1] (no-op on interior cores)
                for (lo, hi, col) in ((mlo, 12, 0), (44, mhi, 1)):
                    if hi <= lo:
                        continue
                    nc.vector.tensor_scalar_mul(
                        view[0:64, lo - base:hi - base, :],
                        view[0:64, lo - base:hi - base, :],
                        mm_t[0:64, col:col + 1])
                if not stacked:
                    return
                # upper half holds rows shifted by +1 slot
                for (lo, hi, col) in ((mlo, 12, 0), (44, mhi, 1)):
                    rlo = max(0, lo - base - 1)
                    rhi = min(nrows, hi - base - 1)
                    if rhi <= rlo:
                        continue
                    nc.vector.tensor_scalar_mul(
                        view[64:128, rlo:rhi, :],
                        view[64:128, rlo:rhi, :],
                        mm_t[64:128, col:col + 1])

            def stack_dma(buf, base, s0, nrows):
                # buf[64:128, r] := buf[0:64, r+1] for the rows enabled by the
                # freshly written tile (slots s0, s0+1)
                d0 = max(0, s0 - base - 1)
                d1 = min(nrows - 1, s0 - base + 1)
                if d1 <= d0:
                    return
                nc.sync.dma_start(
                    buf[64:128, d0 * PITCH:d1 * PITCH],
                    buf[0:64, (d0 + 1) * PITCH:(d1 + 1) * PITCH])

            for _rep in range(reps):
              chunk_idx = 0
              for m in range(G):
                # --- stream this group's tail weights ---
                wt_t = wtp.tile([128, 1152], f32r, tag="wt")
                nc.sync.dma_start(wt_t[:], wt_d[m * 128:(m + 1) * 128, :])

                # --- h_m: K=9 matmuls from streamed r9 chunks ---
                h_lo, h_hi = _h_range(m)
                hdst, hbase = (TAv, 1) if m == 0 else (Fv, 1)
                for s0 in range(h_lo, h_hi, 2):
                    r9c = r9p.tile([9, 2 * 258], f32r, tag="r9")
                    nc.sync.dma_start(
                        r9c[:], r9_d[chunk_idx * 9:(chunk_idx + 1) * 9, :])
                    chunk_idx += 1
                    r9v = r9c[:].rearrange("p (r x) -> p r x", x=258)
                    pt = ps.tile([64, 512], f32, tag="ps")
                    nc.tensor.matmul(pt[:], wh_t[:, m * 64:(m + 1) * 64],
                                     r9v[0:9, 0:2, 1:257], start=True, stop=True)
                    nc.scalar.activation(
                        hdst[0:64, s0 - hbase:s0 - hbase + 2, 1:257], pt[:],
                        PRELU, bias=bb_t[:, m:m + 1], scale=1.0,
                        alpha=aa_t[:, m:m + 1])
                    if m == 0:
                        stack_dma(TA, 1, s0, NSLOT)
                mask(hdst, hbase, h_lo, h_hi, stacked=(m == 0))

                # --- fusion m (m>=1): K=128 from F = [h_m | feature_{m-1}] ---
                if m >= 1:
                    wf_t = wfp.tile([128, 576], f32r, tag="wf")
                    nc.sync.dma_start(
                        wf_t[:], wf_d[(m - 1) * 128:m * 128, :])
                    f_lo, f_hi = _fus_range(m)
                    for s0 in range(f_lo, f_hi, 2):
                        pt = ps.tile([64, 512], f32, tag="ps")
                        for t in range(9):
                            dy, dx = t // 3, t % 3
                            rr = s0 + dy - 1 - 1
                            nc.tensor.matmul(
                                pt[:], wf_t[:, t * 64:(t + 1) * 64],
                                Fv[0:128, rr:rr + 2, dx:dx + 256],
                                start=(t == 0), stop=(t == 8))
                        nc.scalar.activation(
                            TAv[0:64, s0 - 1:s0 + 1, 1:257], pt[:],
                            PRELU, bias=bb_t[:, 8 + m - 1:8 + m],
                            scale=1.0, alpha=aa_t[:, 8 + m - 1:8 + m])
                        stack_dma(TA, 1, s0, NSLOT)
                    mask(TAv, 1, f_lo, f_hi, stacked=True)

                # --- feature_m (in TA) -> F[64:128] for next fusion ---
                if m < G - 1:
                    lo, hi = (1, 55) if m == 0 else _fus_range(m)
                    nc.sync.dma_start(
                        F[64:128, (lo - 1) * PITCH:(hi - 1) * PITCH],
                        TA[0:64, (lo - 1) * PITCH:(hi - 1) * PITCH])

                # --- tails (dy-packed: 3x K=128 + 3x K=64 per tile) ---
                def tconv(src_v, src_base, dst_v, dst_base, dst_buf, dst_rows,
                          lo, hi, cv, bcol):
                    for s0 in range(lo, hi, 2):
                        pt = ps.tile([64, 512], f32, tag="ps")
                        for j in range(6):
                            dx = j % 3
                            c0 = (cv * 6 + j) * 64
                            if j < 3:   # dy=0 (lower) + dy=1 (stacked upper)
                                rr = s0 - 1 - src_base
                                nc.tensor.matmul(
                                    pt[:], wt_t[:, c0:c0 + 64],
                                    src_v[0:128, rr:rr + 2, dx:dx + 256],
                                    start=(j == 0), stop=False)
                            else:       # dy=2 from lower half
                                rr = s0 + 1 - src_base
                                nc.tensor.matmul(
                                    pt[:], wt_t[0:64, c0:c0 + 64],
                                    src_v[0:64, rr:rr + 2, dx:dx + 256],
                                    start=False, stop=(j == 5))
                        nc.scalar.activation(
                            dst_v[0:64, s0 - dst_base:s0 - dst_base + 2, 1:257],
                            pt[:], PRELU, bias=bb_t[:, bcol:bcol + 1],
                            scale=1.0, alpha=aa_t[:, bcol:bcol + 1])
                        stack_dma(dst_buf, dst_base, s0, dst_rows)

                tconv(TAv, 1, TBv, TB_BASE, TB, TB_ROWS, 9, 47, 0, 15 + m)
                mask(TBv, TB_BASE, 9, 47, stacked=True, nrows=TB_ROWS)
                tconv(TBv, TB_BASE, TAv, 1, TA, NSLOT, 10, 46, 1, 23 + m)
                mask(TAv, 1, 10, 46, stacked=True)
                tconv(TAv, 1, TBv, TB_BASE, TB, TB_ROWS, 11, 45, 2, 31 + m)
                mask(TBv, TB_BASE, 11, 45, stacked=True, nrows=TB_ROWS)

                # --- t5: M=1, dy-packed like the tails ---
                for s0 in range(12, 44, 2):
                    pt5 = p5p.tile([1, 512], f32, tag="p5")
                    for j in range(6):
                        dx = j % 3
                        c5 = m * 6 + j
                        if j < 3:
                            rr = s0 - 1 - TB_BASE
                            nc.tensor.matmul(
                                pt5[:], w5_t[:, c5:c5 + 1],
                                TBv[0:128, rr:rr + 2, dx:dx + 256],
                                start=(j == 0), stop=False)
                        else:
                            rr = s0 + 1 - TB_BASE
                            nc.tensor.matmul(
                                pt5[:], w5_t[0:64, c5:c5 + 1],
                                TBv[0:64, rr:rr + 2, dx:dx + 256],
                                start=False, stop=(j == 5))
                    o5 = o5p.tile([1, 512], f32, tag="o5")
                    nc.scalar.activation(o5[:], pt5[:], COPY)
                    nc.sync.dma_start(
                        o_d[m, (s0 - 12) * 256:(s0 - 10) * 256], o5[:])

    nc.compile()
    return nc


def _get_program():
    global _BUILT
    if _BUILT is None:
        _BUILT = _build_program()
    return _BUILT


def _host_heads(x, sample_w, up_w, up_b):
    """r[m] (256x256) for all groups, float32."""
    X = x[0, 0].reshape(8, 32, 8, 32).astype(np.float64)
    R = np.empty((G, H, W), np.float32)
    for m in range(G):
        S = np.einsum('ipjq,cpq->cij', X, sample_w[m, :, 0].astype(np.float64))
        U = np.einsum('cij,uc->uij', S, up_w[m, :, :, 0, 0].astype(np.float64))
        U = U + up_b[m].astype(np.float64)[:, None, None]
        R[m] = U.reshape(32, 32, 8, 8).transpose(2, 0, 3, 1).reshape(256, 256)
    return R


def _build_r9(R):
    """Per-core prestacked h-conv rhs: [8][NCHUNK*9, 516] float32."""
    from numpy.lib.stride_tricks import sliding_window_view
    rp = np.zeros((G, H + 26, W + 4), np.float32)   # rows g+13, cols x+2
    rp[:, 13:13 + H, 2:2 + W] = R
    out = np.empty((8, NCHUNK, 9, 516), np.float32)
    k0 = 0
    for m in range(G):
        lo, hi = _h_range(m)
        s0s = np.arange(lo, hi, 2)
        SW = sliding_window_view(rp[m], (2, 258))
        for t in range(9):
            dy, dx = t // 3, t % 3
            g0 = (32 * np.arange(8))[:, None] + s0s[None, :] + dy
            out[:, k0:k0 + len(s0s), t] = SW[g0, dx].reshape(8, len(s0s), 516)
        k0 += len(s0s)
    return out.reshape(8, NCHUNK * 9, 516)


_EXEC = None


def _get_executor():
    """Persistent jitted shard_map executor over 8 cores (mirrors
    bass2jax.run_bass_via_pjrt, but reusable for repeat timing)."""
    global _EXEC
    if _EXEC is not None:
        return _EXEC
    import jax
    import jax.numpy as jnp
    from jax.sharding import Mesh, PartitionSpec
    from jax.experimental.shard_map import shard_map
    import concourse.mybir as mybir
    from concourse import bass2jax

    nc = _get_program()
    bass2jax.install_neuronx_cc_hook()

    part_name = nc.partition_id_tensor.name if nc.partition_id_tensor else None
    in_names, out_names, out_avals, zero_shapes = [], [], [], []
    for alloc in nc.m.functions[0].allocations:
        if not isinstance(alloc, mybir.MemoryLocationSet):
            continue
        name = alloc.memorylocations[0].name
        if alloc.kind == "ExternalInput":
            if name != part_name:
                in_names.append(name)
        elif alloc.kind == "ExternalOutput":
            out_names.append(name)
            shape = tuple(alloc.tensor_shape)
            dtype = mybir.dt.np(alloc.dtype)
            out_avals.append(jax.core.ShapedArray(shape, dtype))
            zero_shapes.append((shape, dtype))
    n_params = len(in_names)
    all_names = in_names + out_names
    if part_name is not None:
        all_names = all_names + [part_name]

    def _body(*args):
        operands = list(args)
        if part_name is not None:
            operands.append(bass2jax.partition_id_tensor())
        outs = bass2jax._bass_exec_p.bind(
            *operands,
            out_avals=tuple(out_avals),
            in_names=tuple(all_names),
            out_names=tuple(out_names),
            lowering_input_output_aliases=(),
            sim_require_finite=True,
            sim_require_nnan=True,
            nc=nc,
        )
        return tuple(outs)

    devices = jax.devices()[:8]
    mesh = Mesh(np.asarray(devices), ("core",))
    n_outs = len(out_names)
    sharded = jax.jit(
        shard_map(_body, mesh=mesh,
                  in_specs=(PartitionSpec("core"),) * (n_params + n_outs),
                  out_specs=(PartitionSpec("core"),) * n_outs,
                  check_rep=False),
        keep_unused=True)
    _EXEC = (sharded, in_names, out_names, zero_shapes)
    return _EXEC


def _prep_device_args(in_maps):
    import jax
    sharded, in_names, out_names, zero_shapes = _get_executor()
    concat_in = [np.concatenate([in_maps[c][n] for c in range(8)], axis=0)
                 for n in in_names]
    concat_zero = [np.zeros((8 * s[0],) + tuple(s[1:]), d)
                   for (s, d) in zero_shapes]
    return [jax.device_put(a) for a in concat_in + concat_zero]


def _run(in_maps):
    sharded, in_names, out_names, zero_shapes = _get_executor()
    args = _prep_device_args(in_maps)
    outs = sharded(*args)
    res = []
    for c in range(8):
        res.append({n: np.asarray(outs[i]).reshape((8,) + zero_shapes[i][0])[c]
                    for i, n in enumerate(out_names)})
    return res


def bench(in_maps, iters=5):
    """Device-resident repeat timing of the sharded program. Returns
    (best_seconds, times)."""
    import time as _t
    sharded, *_ = _get_executor()
    args = _prep_device_args(in_maps)
    r = sharded(*args)
    [x.block_until_ready() for x in r]
    times = []
    for _ in range(iters):
        t0 = _t.perf_counter()
        r = sharded(*args)
        [x.block_until_ready() for x in r]
        times.append(_t.perf_counter() - t0)
    return min(times), times


def _make_executor(nc):
    import jax
    from jax.sharding import Mesh, PartitionSpec
    from jax.experimental.shard_map import shard_map
    from concourse import bass2jax
    import concourse.mybir as mybir

    bass2jax.install_neuronx_cc_hook()
    part_name = nc.partition_id_tensor.name if nc.partition_id_tensor else None
    in_names, out_names, out_avals, zero_shapes = [], [], [], []
    for alloc in nc.m.functions[0].allocations:
        if not isinstance(alloc, mybir.MemoryLocationSet):
            continue
        name = alloc.memorylocations[0].name
        if alloc.kind == "ExternalInput":
            if name != part_name:
                in_names.append(name)
        elif alloc.kind == "ExternalOutput":
            out_names.append(name)
            shape = tuple(alloc.tensor_shape)
            dtype = mybir.dt.np(alloc.dtype)
            out_avals.append(jax.core.ShapedArray(shape, dtype))
            zero_shapes.append((shape, dtype))
    all_names = in_names + out_names + ([part_name] if part_name else [])

    def _body(*args):
        operands = list(args)
        if part_name:
            operands.append(bass2jax.partition_id_tensor())
        return tuple(bass2jax._bass_exec_p.bind(
            *operands, out_avals=tuple(out_avals), in_names=tuple(all_names),
            out_names=tuple(out_names), lowering_input_output_aliases=(),
            sim_require_finite=True, sim_require_nnan=True, nc=nc))

    mesh = Mesh(np.asarray(jax.devices()[:8]), ("core",))
    n = len(in_names) + len(out_names)
    sharded = jax.jit(shard_map(_body, mesh=mesh,
                                in_specs=(PartitionSpec("core"),) * n,
                                out_specs=(PartitionSpec("core"),) * len(out_names),
                                check_rep=False), keep_unused=True)
    return sharded, in_names, out_names, zero_shapes


def bench_reps(in_maps, iters=5):
    """Time a 2x-unrolled variant of the program against the 1x program;
    the wall-clock difference is one full device execution, free of the
    fixed axon-RPC dispatch overhead (~100ms) that dominates single calls."""
    import time as _t
    import jax
    results = {}
    for reps in (1, 2):
        nc = _get_program() if reps == 1 else _build_program(reps=2)
        sharded, in_names, out_names, zero_shapes = _make_executor(nc)
        concat_in = [np.concatenate([in_maps[c][n] for c in range(8)], axis=0)
                     for n in in_names]
        concat_zero = [np.zeros((8 * s[0],) + tuple(s[1:]), d)
                       for (s, d) in zero_shapes]
        args = [jax.device_put(a) for a in concat_in + concat_zero]
        r = sharded(*args); [x.block_until_ready() for x in r]
        ts = []
        for _ in range(iters):
            t0 = _t.perf_counter()
            r = sharded(*args)
            [x.block_until_ready() for x in r]
            ts.append(_t.perf_counter() - t0)
        ts.sort()
        results[reps] = ts
    # median-based difference is more robust to tunnel jitter than min
    import statistics
    d = statistics.median(results[2]) - statistics.median(results[1])
    return max(d, 0.0), results


def build_in_maps(x, sample_w, up_w, up_b, h1_w, h1_b, h1_a, fus_w, fus_b,
                  fus_a, t2_w, t2_b, t2_a, t3_w, t3_b, t3_a, t4_w, t4_b,
                  t4_a, t5_w, t5_b):

    R = _host_heads(x, sample_w, up_w, up_b)
    r9 = _build_r9(R)

    wh = np.ascontiguousarray(
        h1_w[:, :, 0].reshape(G, 64, 9).transpose(2, 0, 1).reshape(9, G * 64))
    # fusion lhsT rows 0:64 <- h weights (cat idx 64:128), rows 64:128 <- feature
    wf = np.empty((7, 128, 9, 64), np.float32)
    for mm1 in range(7):
        for t in range(9):
            wf[mm1, 0:64, t] = fus_w[mm1, :, 64:128, t // 3, t % 3].T
            wf[mm1, 64:128, t] = fus_w[mm1, :, 0:64, t // 3, t % 3].T
    wf = wf.reshape(7 * 128, 576)
    wt = np.zeros((G, 128, 3, 6, 64), np.float32)
    for m in range(G):
        for cv, tw in enumerate((t2_w, t3_w, t4_w)):
            for dx in range(3):
                wt[m, 0:64, cv, dx] = tw[m, :, :, 0, dx].T
                wt[m, 64:128, cv, dx] = tw[m, :, :, 1, dx].T
                wt[m, 0:64, cv, 3 + dx] = tw[m, :, :, 2, dx].T
    wt = wt.reshape(G * 128, 1152)
    w5 = np.zeros((128, G * 6), np.float32)
    for m in range(G):
        for dx in range(3):
            w5[0:64, m * 6 + dx] = t5_w[m, 0, :, 0, dx]
            w5[64:128, m * 6 + dx] = t5_w[m, 0, :, 1, dx]
            w5[0:64, m * 6 + 3 + dx] = t5_w[m, 0, :, 2, dx]
    bb = np.zeros((64, 39), np.float32)
    aa = np.zeros((64, 39), np.float32)
    bb[:, 0:8] = h1_b.T; aa[:, 0:8] = np.broadcast_to(h1_a, (64, 8))
    bb[:, 8:15] = fus_b.T; aa[:, 8:15] = np.broadcast_to(fus_a, (64, 7))
    bb[:, 15:23] = t2_b.T; aa[:, 15:23] = np.broadcast_to(t2_a, (64, 8))
    bb[:, 23:31] = t3_b.T; aa[:, 23:31] = np.broadcast_to(t3_a, (64, 8))
    bb[:, 31:39] = t4_b.T; aa[:, 31:39] = np.broadcast_to(t4_a, (64, 8))

    in_maps = []
    for c in range(8):
        mmk = np.ones((128, 2), np.float32)
        if c == 0:
            mmk[:, 0] = 0.0
        if c == 7:
            mmk[:, 1] = 0.0
        in_maps.append({"r9": r9[c], "wh": wh, "wf": wf, "wt": wt, "w5": w5,
                        "bb": bb, "aa": aa, "mm": mmk})
    return in_maps


def kernel(x, sample_w, up_w, up_b, h1_w, h1_b, h1_a, fus_w, fus_b, fus_a,
           t2_w, t2_b, t2_a, t3_w, t3_b, t3_a, t4_w, t4_b, t4_a, t5_w, t5_b):
    in_maps = build_in_maps(
        x, sample_w, up_w, up_b, h1_w, h1_b, h1_a, fus_w, fus_b, fus_a,
        t2_w, t2_b, t2_a, t3_w, t3_b, t3_a, t4_w, t4_b, t4_a, t5_w, t5_b)
    results = _run(in_maps)
    out = np.empty((G, 1, 1, H, W), np.float32)
    for c in range(8):
        o = results[c]["o"].reshape(G, 32, 256)
        out[:, 0, 0, 32 * c:32 * c + 32, :] = o
    out += np.asarray(t5_b).reshape(G, 1, 1, 1, 1)
    return out
